# revision 33
# baseline (speedup 1.0000x reference)
"""Bass/Tile kernel for nn_BinaryClassifierChain on 8 trn2 cores (v4).

Math (per reference.py):
  wc   = softmax(word_class_features, axis=0)            # over batch dim
  base = concat([features, wc], -1)                      # [B, W, 1088]
  L    = base @ W[:, :1088].T + b                        # [B, W, 32]
  chain: p_i = sigmoid(L_i + sum_{j<i} Wbin[i, j] p_j)   # Wbin = W[:, 1088:]

Sharding: pure data-parallel over the words dim (1024 = 8 x 128); the
batch-softmax stays intact per shard.

v4 vs v3: the v3 trace showed (a) the wc load starving ~60us behind the
feature stream on the shared SDMA engines, and (b) the gpsimd SWDGE
cast-load path capping at ~145 GB/s.  So:
  - features load as plain f32 on the two HWDGE rings (sync + scalar,
    alternating groups); PE transposes run in f32 (transpose_mode) and
    the psum->SBUF evacuation casts to bf16 for the matmul.
  - wc loads FIRST on the sync ring, before any feature traffic.
  - output stores moved to the now-idle gpsimd SWDGE queue.
  - chain: chunk0 = batches 0-31 interleaved into groups 8-15; tail =
    two interleaved 16-batch half-chains.
"""

import sys

sys.path.insert(0, "/opt/trn_rl_repo")

import numpy as np
import orjson
import ml_dtypes

import concourse.bass as bass
import concourse.mybir as mybir
import concourse.tile as tile
from concourse import masks
from concourse.bass_utils import run_bass_kernel_spmd

F32 = mybir.dt.float32
BF16 = mybir.dt.bfloat16
AF = mybir.ActivationFunctionType
ALU = mybir.AluOpType
AX = mybir.AxisListType

B = 64          # batch
NWALL = 1024    # total words
NCORES = 8
NW = NWALL // NCORES  # 128 words per core
D = 1024        # embed dim
C = 64          # word classes
NB = 32         # bin features
DIN = D + C + NB  # 1120
GRP = 4         # batches per matmul group (4 * 128 words = 512 tokens)
NGRP = B // GRP

CH0 = 32        # chain chunk 0 = batches [0, CH0)


def _split_multiwait_json(raw: bytes) -> bytes:
    """walrus in this container only accepts 1 sync-wait per most
    instructions; Tile's final drain (and some others) carry several.
    Move extras onto preceding EventSemaphore carriers (2 waits each) on
    the same engine."""
    bir = orjson.loads(raw)
    for fn in bir["functions"]:
        for blk in fn["blocks"]:
            out = []
            for ins in blk["instructions"]:
                si = ins.get("sync_info")
                waits = (si or {}).get("on_wait") or []
                if len(waits) > 1:
                    extra = waits[:-1]
                    for k in range(0, len(extra), 2):
                        out.append(
                            {
                                "debug": ins.get("debug", 0),
                                "engine": ins["engine"],
                                "ins": [],
                                "outs": [],
                                "name": f"{ins['name']}_sw{k}",
                                "opcode": "EventSemaphore",
                                "sync_info": {
                                    "on_update": [],
                                    "on_wait": extra[k : k + 2],
                                },
                            }
                        )
                    si["on_wait"] = [waits[-1]]
                out.append(ins)
            blk["instructions"] = out
    return orjson.dumps(bir)


def build_program():
    nc = bass.Bass("TRN2", target_bir_lowering=False, debug=False)

    feat = nc.dram_tensor("feat", [B, NW, D], F32, kind="ExternalInput")
    wc = nc.dram_tensor("wc", [B, NW, C], F32, kind="ExternalInput")
    wtrd = nc.dram_tensor("wtr", [128, 9, NB], BF16, kind="ExternalInput")
    vrd = nc.dram_tensor("vrows", [128, NB, NB], BF16, kind="ExternalInput")
    # b host-tiled to [NB, 128] so its load is 512B-per-partition runs; a
    # [NB, 1] load (4B descriptors) can starve ~40us behind the feature
    # stream and head-of-line block the ACT queue.
    bt = nc.dram_tensor("b", [NB, 128], F32, kind="ExternalInput")
    out = nc.dram_tensor("out", [B, NW, NB], BF16, kind="ExternalOutput")

    with tile.TileContext(nc) as tc:
        with (
            tc.tile_pool(name="const", bufs=1) as constp,
            tc.tile_pool(name="x2", bufs=5) as x2p,
            tc.tile_pool(name="xt", bufs=2) as xtp,
            tc.tile_pool(name="blt", bufs=2) as bltp,
            tc.tile_pool(name="tp", bufs=2, space="PSUM") as tpp,
            tc.tile_pool(name="wcps", bufs=1, space="PSUM") as wcpsp,
            tc.tile_pool(name="mmps", bufs=2, space="PSUM") as mmpsp,
            tc.tile_pool(name="petps", bufs=1, space="PSUM") as petpsp,
        ):
            # wc softmax input load comes FIRST on the sync ring so it is
            # not starved by the feature stream.
            wcs = constp.tile([128, B, C], F32)
            nc.sync.dma_start(wcs[:], wc.ap().rearrange("b p c -> p b c"))

            # ---------------- prep (host-precomputed weights) ----------
            ident = constp.tile([128, 128], BF16)
            masks.make_identity(nc, ident[:])
            identf = constp.tile([128, 128], F32)
            masks.make_identity(nc, identf[:])

            b_sb = constp.tile([NB, 128], F32)
            nc.scalar.dma_start(b_sb[:], bt.ap())
            wtr = constp.tile([128, 9, NB], BF16)
            nc.scalar.dma_start(wtr[:], wtrd.ap())
            vr = constp.tile([128, NB, NB], BF16)
            nc.scalar.dma_start(vr[:], vrd.ap())

            wcn = constp.tile([128, B, C], BF16)
            # token-major chain state: [words, batch, bins]; slot i holds
            # L_i until bin i's sigmoid overwrites it with p_i
            Z = constp.tile([128, B, NB], BF16)
            tmp0 = constp.tile([128, CH0, NB + 1], BF16)
            zc0 = constp.tile([128, CH0], F32)
            BH = (B - CH0) // 2
            tmp1 = constp.tile([128, BH, NB + 1], BF16)
            zc1 = constp.tile([128, BH], F32)
            BQ = BH // 2
            tmp2 = constp.tile([128, BQ, NB + 1], BF16)
            zc2 = constp.tile([128, BQ], F32)
            tmp3 = constp.tile([128, BQ, NB + 1], BF16)
            zc3 = constp.tile([128, BQ], F32)

            # ---------------- softmax over batch ----------------
            with tc.tile_pool(name="soft", bufs=1) as softp:
                ex = softp.tile([128, B, C], F32)
                nc.scalar.activation(ex[:], wcs[:], AF.Exp)
                acc = softp.tile([128, B // 2, C], F32)
                nc.vector.tensor_add(
                    acc[:], ex[:, 0 : B // 2, :], ex[:, B // 2 : B, :]
                )
                h = B // 4
                while h >= 1:
                    nc.vector.tensor_add(
                        acc[:, 0:h, :], acc[:, 0:h, :], acc[:, h : 2 * h, :]
                    )
                    h //= 2
                rec = softp.tile([128, C], F32)
                nc.vector.reciprocal(rec[:], acc[:, 0, :])
                nc.vector.tensor_mul(
                    wcn[:],
                    ex[:],
                    rec[:].unsqueeze(1).broadcast_to([128, B, C]),
                )

            # ---------------- chain helper ----------------
            def chain_bin(i, bs, tmp, zc):
                nbt = bs.stop - bs.start
                if i == 0:
                    nc.scalar.activation(Z[:, bs, 0], Z[:, bs, 0], AF.Sigmoid)
                    return
                nc.vector.tensor_mul(
                    tmp[:, :, 0 : i + 1],
                    Z[:, bs, 0 : i + 1],
                    vr[:, i, 0 : i + 1]
                    .unsqueeze(1)
                    .broadcast_to([128, nbt, i + 1]),
                )
                nc.vector.reduce_sum(zc[:, :], tmp[:, :, 0 : i + 1], axis=AX.X)
                nc.scalar.activation(Z[:, bs, i], zc[:, :], AF.Sigmoid)

            bs0 = slice(0, CH0)
            bsA = slice(CH0, CH0 + BH)
            bsB = slice(CH0 + BH, B)
            CH_SLOT_G0 = 8   # chunk-0 bins spread over groups 8..15

            def c0_bins_for(g, pos):
                if g < CH_SLOT_G0:
                    return []
                base = (g - CH_SLOT_G0) * 4
                return [base + pos] if pos < 4 else []

            def c1a_early(g, pos):
                """first 16 bins of the c1a half-chain (batches 32-47,
                ready after group 11) run during groups 12-15."""
                if g < 12:
                    return
                base = (g - 12) * 4
                if pos == 4:
                    chain_bin(base, bsA, tmp1, zc1)
                    chain_bin(base + 1, bsA, tmp1, zc1)
                else:
                    chain_bin(base + 2, bsA, tmp1, zc1)
                    chain_bin(base + 3, bsA, tmp1, zc1)

            # ---------------- main matmul pipeline ----------------
            for g in range(NGRP):
                b0 = g * GRP
                x2 = x2p.tile([128, GRP, D], F32, tag="x2")
                # each group's 2MB is split across BOTH HWDGE rings so the
                # two rings work the same group concurrently (one ring
                # alone sustains ~270 GB/s; both loaded were measured at
                # ~400 GB/s during startup).
                nc.sync.dma_start(
                    x2[:, 0:2, :],
                    feat.ap()[b0 : b0 + 2, :, :].rearrange("b p d -> p b d"),
                )
                nc.scalar.dma_start(
                    x2[:, 2:4, :],
                    feat.ap()[b0 + 2 : b0 + 4, :, :].rearrange("b p d -> p b d"),
                )
                xts = xtp.tile([128, 9, 512], BF16, tag="xt")
                for kh in range(4):
                    pt = tpp.tile([128, 2, 512], F32, tag="xtps")
                    for kk in range(2):
                        k = kh * 2 + kk
                        for bi in range(GRP):
                            nc.tensor.transpose(
                                pt[:, kk, bi * 128 : (bi + 1) * 128],
                                x2[:, bi, k * 128 : (k + 1) * 128],
                                identf[:],
                            )
                    # cast-evacuation f32 psum -> bf16 SBUF, split ACT/DVE
                    if kh % 2 == 0:
                        nc.scalar.copy(xts[:, kh * 2 : kh * 2 + 2, :], pt[:])
                    else:
                        nc.vector.tensor_copy(xts[:, kh * 2 : kh * 2 + 2, :], pt[:])
                    if kh < 2:
                        for i in c0_bins_for(g, kh):
                            chain_bin(i, bs0, tmp0, zc0)

                # softmaxed wc as 9th k-chunk: transpose on chip
                wps = wcpsp.tile([64, 512], BF16, tag="wct")
                for bi in range(GRP):
                    nc.tensor.transpose(
                        wps[:, bi * 128 : (bi + 1) * 128],
                        wcn[:, b0 + bi, :],
                        ident[:],
                    )
                nc.scalar.copy(xts[0:64, 8, :], wps[:])
                for i in c0_bins_for(g, 2):
                    chain_bin(i, bs0, tmp0, zc0)
                c1a_early(g, 4)

                ps = mmpsp.tile([NB, 512], F32, tag="mm")
                for k in range(8):
                    nc.tensor.matmul(
                        ps[:], wtr[:, k, :], xts[:, k, :],
                        start=(k == 0), stop=False,
                    )
                nc.tensor.matmul(
                    ps[:], wtr[0:64, 8, :], xts[0:64, 8, :],
                    start=False, stop=True,
                )
                blt = bltp.tile([NB, 512], F32, tag="blt")
                nc.scalar.activation(
                    blt[:], ps[:], AF.Identity, bias=b_sb[:, 0:1], scale=1.0
                )
                # corner turn: 4 x [32,128] -> one [128, 4*32] psum, one copy
                ptc = petpsp.tile([128, 128], F32, tag="pet")
                for q in range(GRP):
                    nc.tensor.transpose(
                        ptc[:, q * NB : (q + 1) * NB],
                        blt[:, q * 128 : (q + 1) * 128],
                        identf[0:NB, 0:NB],
                    )
                nc.vector.tensor_copy(Z[:, b0 : b0 + GRP, :], ptc[:])
                for i in c0_bins_for(g, 3):
                    chain_bin(i, bs0, tmp0, zc0)
                c1a_early(g, 5)

            # chunk-0 store (batches 0..CH0) on the idle SWDGE queue
            nc.gpsimd.dma_start(
                out.ap()[0:CH0, :, :].rearrange("b p i -> p b i"), Z[:, bs0, :]
            )

            # ---------------- tail: 3-way interleaved chains ----------
            # c1a (16 batches) resumes at bin 16 (0-15 ran during g12-15);
            # the last 16 batches run as TWO 8-batch chains b1/b2 so that
            # consecutive ops never belong to the same serial chain (a
            # same-chain adjacency head-of-line blocks the in-order DVE
            # queue on the previous sigmoid).  c1a advances every other
            # level so all three chains drain together.
            bsB1 = slice(CH0 + BH, CH0 + BH + BQ)
            bsB2 = slice(CH0 + BH + BQ, B)
            for t in range(NB):
                chain_bin(t, bsB1, tmp2, zc2)
                if t % 2 == 0 and 16 + t // 2 < NB:
                    chain_bin(16 + t // 2, bsA, tmp1, zc1)
                chain_bin(t, bsB2, tmp3, zc3)
                if t % 2 == 1 and 16 + t // 2 == NB - 1:
                    # c1a finished: overlap its store with b1/b2 remainder
                    nc.gpsimd.dma_start(
                        out.ap()[CH0 : CH0 + BH, :, :].rearrange("b p i -> p b i"),
                        Z[:, bsA, :],
                    )
            nc.gpsimd.dma_start(
                out.ap()[CH0 + BH : B, :, :].rearrange("b p i -> p b i"),
                Z[:, CH0 + BH : B, :],
            )

    orig = nc.to_json_bytes
    nc.to_json_bytes = lambda: _split_multiwait_json(orig())
    return nc


_PROG = None


def _get_prog():
    global _PROG
    if _PROG is None:
        _PROG = build_program()
    return _PROG


def _host_weights(W, b):
    """Host-side prep of the tiny weight tensors."""
    W = np.asarray(W, dtype=np.float32)
    wtr = np.zeros((128, 9, NB), dtype=ml_dtypes.bfloat16)
    for k in range(8):
        wtr[:, k, :] = W[:, k * 128 : (k + 1) * 128].T.astype(ml_dtypes.bfloat16)
    wtr[0:64, 8, :] = W[:, D : D + C].T.astype(ml_dtypes.bfloat16)
    wbin = W[:, D + C : DIN]  # [32, 32]
    vr = np.zeros((NB, NB), dtype=np.float32)
    for i in range(NB):
        vr[i, :i] = wbin[i, :i]
        vr[i, i] = 1.0
    vrows = np.broadcast_to(
        vr.astype(ml_dtypes.bfloat16)[None], (128, NB, NB)
    ).copy()
    bt = np.ascontiguousarray(
        np.tile(np.asarray(b, dtype=np.float32)[:, None], (1, 128))
    )
    return wtr, vrows, bt


def kernel(features, word_class_features, W, b, trace=False, tmpdir=None):
    features = np.ascontiguousarray(features, dtype=np.float32)
    word_class_features = np.ascontiguousarray(word_class_features, dtype=np.float32)
    wtr, vrows, bf = _host_weights(W, b)

    nc = _get_prog()
    in_maps = []
    for c in range(NCORES):
        sl = slice(c * NW, (c + 1) * NW)
        in_maps.append(
            {
                "feat": np.ascontiguousarray(features[:, sl, :]),
                "wc": np.ascontiguousarray(word_class_features[:, sl, :]),
                "wtr": wtr,
                "vrows": vrows,
                "b": bf,
            }
        )
    res = run_bass_kernel_spmd(
        nc, in_maps, core_ids=list(range(NCORES)), trace=trace, tmpdir=tmpdir
    )
    outp = np.concatenate(
        [res.results[c]["out"].astype(np.float32) for c in range(NCORES)], axis=1
    )
    kernel._last_result = res
    return outp
